# revision 8
# baseline (speedup 1.0000x reference)
"""Phase multi-head attention kernel for Trainium2 (Bass/Tile), 8-core SPMD.

Math (per batch b, head h, with state s = sr + i*si reshaped to (S, HD)):
    q = s * e^{i*q_rot},  k = s * e^{i*k_rot},  v = s * e^{i*v_rot}
    logits[s,t] = Re(q_s . conj(k_t)) = Re(s_s . conj(s_t) e^{i(q_rot-k_rot)})
Only K needs the (q_rot - k_rot) rotation:
    logits = [sr|si] @ (Mt @ [srT;siT]),   Mt = [[C,S],[-S,C]], phi = q_rot-k_rot
The softmax scale cancels (1/sqrt(HD) * 8.0 with HD=64), so attn = softmax
over t<=s of raw logits.  The V rotation is applied after the attention
product: out_r = cos(v)*U1 - sin(v)*U2, out_i = sin(v)*U1 + cos(v)*U2 where
[U1|U2] = attn @ [sr|si].

Sharding: head-parallel, core c owns head c and both batches (2 pairs/core).

Device pipeline per pair (all matmuls 16-bit so FWL stays enabled):
  nat   (128, 16, 128) f32   chunk n holds rows s=n*128+p, cols [sr|si]
  nat16 (128, 16, 128) fp16  QK-side precision (fp16 ~ 2e-3 rel err)
  natb  (128, 16, 130) bf16  [sr|si|1|pad] - PV rhs + denominator column
                             (bf16 for range: exp values reach ~1e19)
  sT    (128, 2048)    fp16  d-stack major, via 16 PE transposes of nat16
  kT    (128, 2048)    fp16  Mt @ sT via 4 matmuls
  Per 512-wide query block j: logitsT (t-chunk, sq) fp16 matmuls -> exp
  (ScalarE, bf16 out) -> causal mask on the diagonal sub-chunk -> PV
  accumulation psum[sq, 129] over t-chunks (bf16); col 128 = denominator.
  Drain: reciprocal + per-chunk scale into contiguous U1/U2 planes, then
  V-rotation (contiguous DVE tensor ops) and one DMA per r/i plane.
"""

import numpy as np

import concourse.bass as bass
import concourse.bacc as bacc
import concourse.mybir as mybir
import concourse.tile as tile
from concourse.masks import make_identity
from concourse.bass_utils import run_bass_kernel_spmd

B, S, D = 2, 2048, 512
H, HD = 8, 64
P = 128
NCHUNK = S // P      # 16 seq chunks of 128
NBLK = 4             # query blocks of 512
BLKW = 512
NATW = 130           # [sr(64) | si(64) | ones(1) | pad(1)]

f32 = mybir.dt.float32
f16 = mybir.dt.float16
bf16 = mybir.dt.bfloat16
EXP = mybir.ActivationFunctionType.Exp


def build_kernel():
    nc = bacc.Bacc("TRN2", target_bir_lowering=False)

    nat_d = [nc.dram_tensor(f"nat{p}", (P, NCHUNK, P), f32, kind="ExternalInput")
             for p in range(B)]
    mtT_d = nc.dram_tensor("mtT", (P, P), f32, kind="ExternalInput")
    cosv_d = nc.dram_tensor("cosv", (1, NCHUNK * HD), f32, kind="ExternalInput")
    sinv_d = nc.dram_tensor("sinv", (1, NCHUNK * HD), f32, kind="ExternalInput")
    out_d = [nc.dram_tensor(f"out{p}", (P, 2, NCHUNK, HD), f32, kind="ExternalOutput")
             for p in range(B)]

    with tile.TileContext(nc) as tc:
        with (
            tc.tile_pool(name="persist", bufs=1) as persist,
            tc.tile_pool(name="work", bufs=4) as work,
            tc.tile_pool(name="pwork", bufs=3, space="PSUM") as pwork,
            tc.tile_pool(name="pout", bufs=2, space="PSUM") as pout,
        ):
            # ---- input DMAs first (nat feeds the PE-critical path) ----
            nats = [persist.tile([P, NCHUNK, P], f32, tag=f"nat{p}", name=f"nat{p}")
                    for p in range(B)]
            for p in range(B):
                for g in range(4):
                    nc.sync.dma_start(out=nats[p][:, g * 4:(g + 1) * 4, :],
                                      in_=nat_d[p][:, g * 4:(g + 1) * 4, :])
            mtT_f = persist.tile([P, P], f32, tag="mtT_f")
            nc.sync.dma_start(out=mtT_f, in_=mtT_d[:, :])
            cosv = persist.tile([P, NCHUNK * HD], f32, tag="cosv")
            nc.sync.dma_start(out=cosv, in_=cosv_d[:, :].to_broadcast((P, NCHUNK * HD)))
            sinv = persist.tile([P, NCHUNK * HD], f32, tag="sinv")
            nc.sync.dma_start(out=sinv, in_=sinv_d[:, :].to_broadcast((P, NCHUNK * HD)))

            ident = persist.tile([P, P], f16, tag="ident")
            make_identity(nc, ident)
            mtT = persist.tile([P, P], f16, tag="mtT")
            nc.vector.tensor_copy(out=mtT, in_=mtT_f)

            # ---- per-pair setup: fp16 cast, transpose, K rotation ----
            nat16s, natbs, sTs, kTs = [], [], [], []
            for p in range(B):
                nat16 = persist.tile([P, NCHUNK, P], f16, tag=f"nat16{p}")
                sT = persist.tile([P, S], f16, tag=f"sT{p}")
                for g in range(4):
                    nc.gpsimd.tensor_copy(out=nat16[:, g * 4:(g + 1) * 4, :],
                                          in_=nats[p][:, g * 4:(g + 1) * 4, :])
                    ps = pwork.tile([P, BLKW], f16, tag="work")
                    for cc in range(4):
                        nc.tensor.transpose(ps[:, cc * P:(cc + 1) * P],
                                            nat16[:, g * 4 + cc, :], ident)
                    nc.vector.tensor_copy(out=sT[:, g * BLKW:(g + 1) * BLKW], in_=ps)

                kT = persist.tile([P, S], f16, tag=f"kT{p}")
                for g in range(4):
                    ps = pwork.tile([P, BLKW], f32, tag="work")
                    nc.tensor.matmul(ps, lhsT=mtT,
                                     rhs=sT[:, g * BLKW:(g + 1) * BLKW],
                                     start=True, stop=True)
                    nc.vector.tensor_copy(out=kT[:, g * BLKW:(g + 1) * BLKW], in_=ps)

                natb = persist.tile([P, NCHUNK, NATW], bf16, tag=f"natb{p}")
                nc.gpsimd.tensor_copy(out=natb[:, :, 0:P], in_=nats[p])
                nc.gpsimd.memset(natb[:, :, P:P + 1], 1.0)

                nat16s.append(nat16)
                natbs.append(natb)
                sTs.append(sT)
                kTs.append(kT)

            # ---- attention + output stage, per pair ----
            for p in range(B):
                u1 = persist.tile([P, NCHUNK * HD], f32, tag=f"u1_{p}")
                u2 = persist.tile([P, NCHUNK * HD], f32, tag=f"u2_{p}")
                for j in range(NBLK):
                    pos = [pout.tile([P, 2, BLKW], f32, tag="pout", name=f"po{p}_{j}_{h2}")
                           for h2 in range(2)]
                    for c in range(4 * j + 4):
                        off = (c - 4 * j) * P if c >= 4 * j else 0
                        qkoff = off if off in (128, 256) else 0
                        psl = pwork.tile([P, BLKW], f32, tag="work")
                        nc.tensor.matmul(
                            psl[:, qkoff:],
                            lhsT=kTs[p][:, c * P:(c + 1) * P],
                            rhs=sTs[p][:, j * BLKW + qkoff:(j + 1) * BLKW],
                            start=True, stop=True)
                        ex = work.tile([P, BLKW], bf16, tag="ex")
                        nc.scalar.activation(out=ex[:, off:], in_=psl[:, off:], func=EXP)
                        if c >= 4 * j:
                            k0 = c - 4 * j
                            # keep where sq_local >= t_local (causal diagonal)
                            nc.gpsimd.affine_select(
                                out=ex[:, k0 * P:(k0 + 1) * P],
                                in_=ex[:, k0 * P:(k0 + 1) * P],
                                compare_op=mybir.AluOpType.is_ge, fill=0.0,
                                base=0, pattern=[[1, P]], channel_multiplier=-1)
                        for k in range(4):
                            if 4 * j + k >= c:
                                nc.tensor.matmul(
                                    pos[k // 2][:, k % 2, 0:P + 1],
                                    lhsT=ex[:, k * P:(k + 1) * P],
                                    rhs=natbs[p][:, c, 0:P + 1],
                                    start=(c == 0), stop=(c == 4 * j + k))
                    for half in range(2):
                        rec = work.tile([P, 2], f32, tag="rec")
                        nc.vector.reciprocal(out=rec, in_=pos[half][:, :, P])
                        for kk in range(2):
                            m = 4 * j + half * 2 + kk
                            nc.vector.tensor_scalar_mul(
                                out=u1[:, m * HD:(m + 1) * HD],
                                in0=pos[half][:, kk, 0:HD], scalar1=rec[:, kk:kk + 1])
                            nc.vector.tensor_scalar_mul(
                                out=u2[:, m * HD:(m + 1) * HD],
                                in0=pos[half][:, kk, HD:P], scalar1=rec[:, kk:kk + 1])
                # V rotation on contiguous planes
                ofr = persist.tile([P, NCHUNK * HD], f32, tag=f"ofr{p}")
                ofi = persist.tile([P, NCHUNK * HD], f32, tag=f"ofi{p}")
                t1 = work.tile([P, NCHUNK * HD], f32, tag="t1")
                t2 = work.tile([P, NCHUNK * HD], f32, tag="t2")
                nc.vector.tensor_mul(t1, u1, cosv)
                nc.gpsimd.tensor_mul(t2, u2, sinv)
                nc.vector.tensor_sub(ofr, t1, t2)
                t3 = work.tile([P, NCHUNK * HD], f32, tag="t3")
                t4 = work.tile([P, NCHUNK * HD], f32, tag="t4")
                nc.gpsimd.tensor_mul(t3, u1, sinv)
                nc.vector.tensor_mul(t4, u2, cosv)
                nc.vector.tensor_add(ofi, t3, t4)
                nc.sync.dma_start(out=out_d[p][:, 0],
                                  in_=ofr.rearrange("p (n d) -> p n d", d=HD))
                nc.sync.dma_start(out=out_d[p][:, 1],
                                  in_=ofi.rearrange("p (n d) -> p n d", d=HD))

    nc.compile()
    return nc


def make_in_maps(state_real, state_imag, q_rot, k_rot, v_rot):
    """Per-core input dicts: core c gets head c, both batches."""
    in_maps = []
    for c in range(H):
        phi = (q_rot[c] - k_rot[c]).astype(np.float32)
        Cp, Sp = np.cos(phi), np.sin(phi)
        mtT = np.block([[np.diag(Cp), np.diag(-Sp)],
                        [np.diag(Sp), np.diag(Cp)]]).astype(np.float32)
        cv = np.tile(np.cos(v_rot[c]).astype(np.float32), NCHUNK)[None, :]
        sv = np.tile(np.sin(v_rot[c]).astype(np.float32), NCHUNK)[None, :]
        m = {"mtT": np.ascontiguousarray(mtT),
             "cosv": np.ascontiguousarray(cv),
             "sinv": np.ascontiguousarray(sv)}
        for p in range(B):
            nat = np.concatenate(
                [state_real[p, :, c * HD:(c + 1) * HD],
                 state_imag[p, :, c * HD:(c + 1) * HD]], axis=1)  # (S, 128)
            natp = nat.reshape(NCHUNK, P, P).transpose(1, 0, 2)  # (128, 16, 128)
            m[f"nat{p}"] = np.ascontiguousarray(natp.astype(np.float32))
        in_maps.append(m)
    return in_maps


def assemble_output(results):
    """results: list of 8 dicts with out0/out1 (128, 2, 16, 64) f32 [r|i planes]."""
    out = np.zeros((B, S, D), dtype=np.complex64)
    for c in range(H):
        for p in range(B):
            o = results[c][f"out{p}"]                       # (128, 2, 16, 64)
            r = o[:, 0].transpose(1, 0, 2).reshape(S, HD)   # (2048, 64)
            i = o[:, 1].transpose(1, 0, 2).reshape(S, HD)
            out[p, :, c * HD:(c + 1) * HD] = r + 1j * i
    return out


_NC_CACHE = []


def kernel(state_real, state_imag, q_rot, k_rot, v_rot):
    state_real = np.asarray(state_real, dtype=np.float32)
    state_imag = np.asarray(state_imag, dtype=np.float32)
    q_rot = np.asarray(q_rot, dtype=np.float32)
    k_rot = np.asarray(k_rot, dtype=np.float32)
    v_rot = np.asarray(v_rot, dtype=np.float32)

    if not _NC_CACHE:
        _NC_CACHE.append(build_kernel())
    nc = _NC_CACHE[0]

    in_maps = make_in_maps(state_real, state_imag, q_rot, k_rot, v_rot)
    res = run_bass_kernel_spmd(nc, in_maps, core_ids=list(range(H)))
    return assemble_output(res.results)


if __name__ == "__main__":
    rng = np.random.default_rng(0)
    inputs = {
        "state_real": rng.standard_normal((B, S, D), dtype=np.float32),
        "state_imag": rng.standard_normal((B, S, D), dtype=np.float32),
        "q_rot": rng.uniform(-np.pi, np.pi, (H, HD)).astype(np.float32),
        "k_rot": rng.uniform(-np.pi, np.pi, (H, HD)).astype(np.float32),
        "v_rot": rng.uniform(-np.pi, np.pi, (H, HD)).astype(np.float32),
    }
    out = kernel(**inputs)
    print("ran:", out.shape, out.dtype)
